# revision 1
# baseline (speedup 1.0000x reference)
"""Grouped-query attention (GQA) Trainium2 Bass kernel.

Problem: B=2, S=2048, DIM=2048, HQ=32, HKV=8, HEAD_DIM=64, causal mask.
Sharding: 8 cores = 2 (batch) x 4 (kv-head groups). Core c handles batch
c//4 and kv-block c%4 (2 kv heads, 8 q heads). Wq/Wk/Wv sharded
column-wise, Wo row-wise; each core writes a partial [S, DIM] output;
host sums the 4 partials per batch and adds bo.

On-chip dataflow (per core, all matmuls bf16 with fp32 PSUM accum):
  - q/k/v loaded TRANSPOSED from HBM via strided APs -> [d, s] tiles,
    cast to bf16 on DVE.
  - GEMM1: qxT[c,s] (Wq stationary), kxT[ck,s], vxT[ck,s]; v then
    PE-transposed to natural vx[j,hd] and packed with a ones column
    (flash-attention denominator trick).
  - GEMM2: scoresT[j,i] = kxT_h^T @ qxT_h, exp on ACT (no max
    subtraction -- scores are O(5) bounded), causal triangular mask
    applied multiplicatively post-exp on diagonal blocks only;
    j-blocks above the diagonal are skipped entirely.
  - GEMM3: attnT[c,i] (+denominator row) = vx1^T @ expT, accumulated
    over j-blocks in PSUM.
  - Normalize via reciprocal + SBUF broadcast-replicate DMA + DVE mul.
  - GEMM4: out[i,e] = attnT^T @ Wo_shard, written as fp32 partial.
"""

import numpy as np
import ml_dtypes

import concourse.bass as bass
import concourse.mybir as mybir
from concourse import bacc
from concourse.tile import TileContext
from concourse.bass_utils import run_bass_kernel_spmd

F32 = mybir.dt.float32
BF16 = mybir.dt.bfloat16
AF = mybir.ActivationFunctionType
ALU = mybir.AluOpType

B, S, DIM = 2, 2048, 2048
HQ, HKV, HD = 32, 8, 64
GROUP = HQ // HKV              # 4
NCORES = 8
KVSH = 4                       # kv-blocks (shards) per batch
CQ = (HQ // KVSH) * HD         # 512 q-proj cols per core (8 heads)
CK = (HKV // KVSH) * HD        # 128 kv-proj cols per core (2 heads)
NDC = DIM // 128               # 16 contraction chunks
NSS = S // 512                 # 4 sequence chunks of 512


def _t_ap(t, s0, d0, np_, nf):
    """AP reading DRAM [S, DIM] tensor transposed: partition=d (np_ rows
    at col d0), free=s (nf rows at row s0)."""
    base = t[0:1, 0:1]
    return bass.AP(tensor=base.tensor, offset=s0 * DIM + d0,
                   ap=[[1, np_], [DIM, nf]])


def _bcast_ap(ap, n):
    """Broadcast a [1, F] AP across n partitions (stride-0 partition)."""
    return bass.AP(tensor=ap.tensor, offset=ap.offset,
                   ap=[[0, n]] + list(ap.ap[1:]))


def build_nc(mode="causal"):
    nc = bacc.Bacc("TRN2", target_bir_lowering=False)

    q = nc.dram_tensor("q", [S, DIM], F32, kind="ExternalInput")
    k = nc.dram_tensor("k", [S, DIM], F32, kind="ExternalInput")
    v = nc.dram_tensor("v", [S, DIM], F32, kind="ExternalInput")
    wq = nc.dram_tensor("wq", [DIM, CQ], F32, kind="ExternalInput")
    wk = nc.dram_tensor("wk", [DIM, CK], F32, kind="ExternalInput")
    wv = nc.dram_tensor("wv", [DIM, CK], F32, kind="ExternalInput")
    wo = nc.dram_tensor("wo", [CQ, DIM], F32, kind="ExternalInput")
    bq = nc.dram_tensor("bq", [CQ], F32, kind="ExternalInput")
    bk = nc.dram_tensor("bk", [CK], F32, kind="ExternalInput")
    bv = nc.dram_tensor("bv", [CK], F32, kind="ExternalInput")
    tri = nc.dram_tensor("tri", [128, 128], BF16, kind="ExternalInput")
    ident = nc.dram_tensor("ident", [128, 128], BF16, kind="ExternalInput")
    mbias = None
    if mode == "dense":
        mbias = nc.dram_tensor("mbias", [S, S], F32, kind="ExternalInput")
    out = nc.dram_tensor("out", [S, DIM], F32, kind="ExternalOutput")

    with TileContext(nc) as tc:
        with (
            tc.tile_pool(name="consts", bufs=1) as consts,
            tc.tile_pool(name="w", bufs=1) as wpool,
            tc.tile_pool(name="wst", bufs=2) as wst,
            tc.tile_pool(name="stg", bufs=2) as stg,
            tc.tile_pool(name="xt", bufs=1) as xt,
            tc.tile_pool(name="acts", bufs=1) as acts,
            tc.tile_pool(name="exp", bufs=3) as expp,
            tc.tile_pool(name="nrm", bufs=2) as nrmp,
            tc.tile_pool(name="ob", bufs=2) as obp,
            tc.tile_pool(name="nat", bufs=1) as natp,
            tc.tile_pool(name="dr", bufs=2, space="DRAM") as drp,
            tc.tile_pool(name="ps2", bufs=2, space="PSUM") as ps2,
            tc.tile_pool(name="ps1", bufs=1, space="PSUM") as ps1,
        ):
            # ---- constants ----
            tri_t = consts.tile([128, 128], BF16, tag="tri")
            nc.gpsimd.dma_start(out=tri_t[:, :], in_=tri[:, :])
            id_t = consts.tile([128, 128], BF16, tag="id")
            nc.gpsimd.dma_start(out=id_t[:, :], in_=ident[:, :])
            bq_t = consts.tile([128, 4], F32, tag="bq")
            nc.gpsimd.dma_start(
                out=bq_t[:, :],
                in_=bass.AP(tensor=bq[0:1].tensor, offset=0,
                            ap=[[1, 128], [128, 4]]))
            bk_t = consts.tile([128, 1], F32, tag="bk")
            nc.gpsimd.dma_start(
                out=bk_t[:, :],
                in_=bass.AP(tensor=bk[0:1].tensor, offset=0,
                            ap=[[1, 128], [128, 1]]))
            bv_rep = consts.tile([128, 128], F32, tag="bv")
            nc.gpsimd.dma_start(
                out=bv_rep[:, :],
                in_=bass.AP(tensor=bv[0:1].tensor, offset=0,
                            ap=[[0, 128], [1, 128]]))

            # ---- weights: load fp32, cast to bf16 ----
            wq_bf, wk_bf, wv_bf, wo_bf = [], [], [], []
            for dc in range(NDC):
                st = stg.tile([128, 512], F32, tag="stg")
                nc.gpsimd.dma_start(out=st[:, :],
                                  in_=wq[dc * 128:(dc + 1) * 128, :])
                t = wpool.tile([128, CQ], BF16, tag=f"wq{dc}")
                nc.vector.tensor_copy(t[:, :], st[:, :])
                wq_bf.append(t)
            for dc in range(NDC):
                st = wst.tile([128, 256], F32, tag="wkv")
                nc.gpsimd.dma_start(out=st[:, 0:128],
                                  in_=wk[dc * 128:(dc + 1) * 128, :])
                nc.gpsimd.dma_start(out=st[:, 128:256],
                                  in_=wv[dc * 128:(dc + 1) * 128, :])
                tk = wpool.tile([128, CK], BF16, tag=f"wk{dc}")
                nc.vector.tensor_copy(tk[:, :], st[:, 0:128])
                wk_bf.append(tk)
                tv = wpool.tile([128, CK], BF16, tag=f"wv{dc}")
                nc.vector.tensor_copy(tv[:, :], st[:, 128:256])
                wv_bf.append(tv)
            for cc in range(4):
                t = wpool.tile([128, DIM], BF16, tag=f"wo{cc}")
                for hf in range(2):
                    st = wst.tile([128, 1024], F32, tag="wo")
                    nc.gpsimd.dma_start(
                        out=st[:, :],
                        in_=wo[cc * 128:(cc + 1) * 128,
                               hf * 1024:(hf + 1) * 1024])
                    nc.vector.tensor_copy(t[:, hf * 1024:(hf + 1) * 1024],
                                          st[:, :])
                wo_bf.append(t)

            # ---- persistent activations ----
            qxT = [acts.tile([128, S], BF16, tag=f"qx{cc}", name=f"qx{cc}") for cc in range(4)]
            kxT = acts.tile([128, S], BF16, tag="kx", name="kx")
            vxT = acts.tile([128, S], BF16, tag="vx", name="vx")
            attnT = [acts.tile([128, S], BF16, tag=f"at{cc}", name=f"at{cc}") for cc in range(4)]
            vx1 = [acts.tile([128, 130], BF16, tag=f"vp{sc}", name=f"vp{sc}")
                   for sc in range(S // 128)]

            for ss in range(NSS):
                s0 = ss * 512
                # ---- natural loads + bf16 casts + PE transpose ----
                qT, kT, vT = [], [], []
                for (src, lst, nm) in ((q, qT, "q"), (k, kT, "k"),
                                       (v, vT, "v")):
                    nats = []
                    for r in range(4):
                        st = stg.tile([128, 2048], F32, tag="stg")
                        nc.gpsimd.dma_start(
                            out=st[:, :],
                            in_=src[s0 + r * 128:s0 + (r + 1) * 128, :])
                        nb = natp.tile([128, 2048], BF16, tag=f"nb{r}",
                                       name=f"nb{r}")
                        nc.vector.tensor_copy(nb[:, :], st[:, :])
                        nats.append(nb)
                    for dc in range(NDC):
                        tp = ps2.tile([128, 512], BF16, tag="tp")
                        for r in range(4):
                            nc.tensor.transpose(
                                tp[:, r * 128:(r + 1) * 128],
                                nats[r][:, dc * 128:(dc + 1) * 128],
                                id_t[:, :])
                        t = xt.tile([128, 512], BF16, tag=f"{nm}T{dc}",
                                    name=f"{nm}T{dc}")
                        nc.vector.tensor_copy(t[:, :], tp[:, :])
                        lst.append(t)

                # ---- GEMM1: projections ----
                for cc in range(4):
                    ps = ps2.tile([128, 512], F32, tag="g1")
                    for dc in range(NDC):
                        nc.tensor.matmul(
                            ps[:, :], wq_bf[dc][:, cc * 128:(cc + 1) * 128],
                            qT[dc][:, :], start=(dc == 0), stop=(dc == NDC - 1))
                    nc.scalar.activation(qxT[cc][:, s0:s0 + 512], ps[:, :],
                                         AF.Identity, bias=bq_t[:, cc:cc + 1])
                ps = ps2.tile([128, 512], F32, tag="g1")
                for dc in range(NDC):
                    nc.tensor.matmul(ps[:, :], wk_bf[dc][:, :], kT[dc][:, :],
                                     start=(dc == 0), stop=(dc == NDC - 1))
                nc.scalar.activation(kxT[:, s0:s0 + 512], ps[:, :],
                                     AF.Identity, bias=bk_t[:, 0:1])
                ps = ps2.tile([128, 512], F32, tag="g1")
                for dc in range(NDC):
                    nc.tensor.matmul(ps[:, :], wv_bf[dc][:, :], vT[dc][:, :],
                                     start=(dc == 0), stop=(dc == NDC - 1))
                nc.scalar.activation(vxT[:, s0:s0 + 512], ps[:, :], AF.Copy)

                # ---- v: PE transpose to natural + ones column ----
                vtp = ps2.tile([128, 512], BF16, tag="tp")
                for sc in range(4):
                    nc.tensor.transpose(
                        vtp[:, sc * 128:(sc + 1) * 128],
                        vxT[:, s0 + sc * 128:s0 + (sc + 1) * 128],
                        id_t[:, :])
                for sc in range(4):
                    jb = ss * 4 + sc
                    vx = vx1[jb]
                    for h2 in range(2):
                        nc.vector.tensor_tensor(
                            vx[:, h2 * 65:h2 * 65 + 64],
                            vtp[:, sc * 128 + h2 * 64:sc * 128 + (h2 + 1) * 64],
                            bv_rep[:, h2 * 64:(h2 + 1) * 64], ALU.add)
                    nc.vector.memset(vx[:, 64:65], 1.0)
                    nc.vector.memset(vx[:, 129:130], 1.0)

                # ---- attention for i-block [s0, s0+512) ----
                njb = 4 * (ss + 1) if mode == "causal" else S // 128
                for h in range(8):
                    # head h lives in tile h%4 at partition (h//4)*64, so its
                    # partition base always equals its kv head's base in kxT
                    # (matmul requires equal base partitions). Host permutes
                    # Wq columns / Wo rows to match this layout.
                    th, po, kv = h % 4, (h // GROUP) * 64, h // GROUP
                    at = ps1.tile([65, 512], F32, tag="at")
                    for jb in range(njb):
                        j0 = jb * 128
                        off = max(0, j0 - s0) if mode == "causal" else 0
                        N = 512 - off
                        sp = ps2.tile([128, 512], F32, tag="sc")
                        nc.tensor.matmul(
                            sp[:, :N],
                            kxT[kv * 64:(kv + 1) * 64, j0:j0 + 128],
                            qxT[th][po:po + 64, s0 + off:s0 + 512],
                            start=True, stop=True)
                        if mode == "dense":
                            mb = nrmp.tile([128, 512], F32, tag="mb")
                            nc.gpsimd.dma_start(
                                out=mb[:, :N],
                                in_=mbias[j0:j0 + 128, s0 + off:s0 + 512])
                            nc.vector.tensor_tensor(sp[:, :N], sp[:, :N],
                                                    mb[:, :N], ALU.add)
                        ex = expp.tile([128, 512], BF16, tag="exp")
                        nc.scalar.activation(ex[:, :N], sp[:, :N], AF.Exp,
                                             scale=0.125)
                        if mode == "causal" and j0 >= s0:
                            nc.vector.tensor_tensor(ex[:, 0:128], ex[:, 0:128],
                                                    tri_t[:, :], ALU.mult)
                        nc.tensor.matmul(
                            at[:, off:512], vx1[jb][:, kv * 65:kv * 65 + 65],
                            ex[:, :N], start=(jb == 0), stop=(jb == njb - 1))
                    # normalize by denominator row (64) and store bf16
                    nm = nrmp.tile([65, 512], F32, tag="nrm")
                    nc.vector.reciprocal(nm[64:65, :], at[64:65, :])
                    dr = drp.tile([1, 512], F32, tag="dn")
                    nc.gpsimd.dma_start(out=dr[0:1, :], in_=nm[64:65, :])
                    nc.gpsimd.dma_start(out=nm[0:64, :],
                                        in_=_bcast_ap(dr[0:1, :], 64))
                    nc.vector.tensor_tensor(
                        attnT[th][po:po + 64, s0:s0 + 512],
                        at[0:64, :], nm[0:64, :], ALU.mult)

                # ---- GEMM4: output projection (partial) ----
                for sc in range(4):
                    i0 = s0 + sc * 128
                    for hf in range(2):
                        ob = obp.tile([128, 1024], F32, tag="ob")
                        for e2 in range(2):
                            ec = hf * 2 + e2
                            g4 = ps1.tile([128, 512], F32, tag="g4")
                            for cc2 in range(4):
                                nc.tensor.matmul(
                                    g4[:, :], attnT[cc2][:, i0:i0 + 128],
                                    wo_bf[cc2][:, ec * 512:(ec + 1) * 512],
                                    start=(cc2 == 0), stop=(cc2 == 3))
                            nc.scalar.activation(
                                ob[:, e2 * 512:(e2 + 1) * 512], g4[:, :],
                                AF.Copy)
                        nc.gpsimd.dma_start(
                            out=out[i0:i0 + 128, hf * 1024:(hf + 1) * 1024],
                            in_=ob[:, :])
    nc.finalize()
    return nc


_CACHE = {}


def _get_nc(mode):
    if mode not in _CACHE:
        _CACHE[mode] = build_nc(mode)
    return _CACHE[mode]


def kernel(q, k, v, mask, Wq, bq, Wk, bk, Wv, bv, Wo, bo):
    q = np.asarray(q, np.float32)
    k = np.asarray(k, np.float32)
    v = np.asarray(v, np.float32)
    mask = np.asarray(mask)
    Wq = np.asarray(Wq, np.float32)
    Wk = np.asarray(Wk, np.float32)
    Wv = np.asarray(Wv, np.float32)
    Wo = np.asarray(Wo, np.float32)
    bq = np.asarray(bq, np.float32)
    bk = np.asarray(bk, np.float32)
    bv = np.asarray(bv, np.float32)
    bo = np.asarray(bo, np.float32)

    m = mask.astype(np.float64)
    if np.array_equal(m, np.tril(np.ones((S, S)))):
        mode = "causal"
    elif np.all(m == 1):
        mode = "none"
    else:
        mode = "dense"

    nc = _get_nc(mode)
    tri_np = np.triu(np.ones((128, 128))).astype(ml_dtypes.bfloat16)
    id_np = np.eye(128).astype(ml_dtypes.bfloat16)

    # On-chip layout places local q head h in tile h%4 at partition
    # (h//4)*64 so q/k partition bases match in the scores matmul. Permute
    # Wq columns / Wo rows / bq accordingly: tile cc holds heads (cc, cc+4).
    head_perm = [h for cc in range(4) for h in (cc, cc + 4)]
    col_perm = np.concatenate(
        [np.arange(h * HD, (h + 1) * HD) for h in head_perm])

    in_maps = []
    for core in range(NCORES):
        b, kb = core // KVSH, core % KVSH
        wq_sh = Wq[:, kb * CQ:(kb + 1) * CQ][:, col_perm]
        wo_sh = Wo[kb * CQ:(kb + 1) * CQ, :][col_perm, :]
        bq_sh = bq[kb * CQ:(kb + 1) * CQ][col_perm]
        im = {
            "q": np.ascontiguousarray(q[b]),
            "k": np.ascontiguousarray(k[b]),
            "v": np.ascontiguousarray(v[b]),
            "wq": np.ascontiguousarray(wq_sh),
            "wk": np.ascontiguousarray(Wk[:, kb * CK:(kb + 1) * CK]),
            "wv": np.ascontiguousarray(Wv[:, kb * CK:(kb + 1) * CK]),
            "wo": np.ascontiguousarray(wo_sh),
            "bq": np.ascontiguousarray(bq_sh),
            "bk": np.ascontiguousarray(bk[kb * CK:(kb + 1) * CK]),
            "bv": np.ascontiguousarray(bv[kb * CK:(kb + 1) * CK]),
            "tri": tri_np,
            "ident": id_np,
        }
        if mode == "dense":
            with np.errstate(divide="ignore"):
                bias = -(1.0 / mask.astype(np.float32) + 1.0)
            im["mbias"] = np.ascontiguousarray(bias.T * 8.0)
        in_maps.append(im)

    res = run_bass_kernel_spmd(nc, in_maps, core_ids=list(range(NCORES)))
    outs = [r["out"] for r in res.results]
    full = np.empty((B, S, DIM), np.float32)
    for b in range(B):
        acc = outs[b * KVSH].astype(np.float32)
        for kb in range(1, KVSH):
            acc = acc + outs[b * KVSH + kb]
        full[b] = acc + bo[None, :]
    return full



# revision 4
# speedup vs baseline: 1.4824x; 1.4824x over previous
"""Grouped-query attention (GQA) Trainium2 Bass kernel, v2.

Problem: B=2, S=2048, DIM=2048, HQ=32, HKV=8, HEAD_DIM=64, causal mask.
Sharding: 8 cores = 2 (batch) x 4 (kv-head groups). Core c handles batch
c//4 and kv-block c%4 (2 kv heads, 8 q heads). Wq/Wk/Wv sharded
column-wise, Wo row-wise; each core writes a partial [S, DIM] output;
host sums the 4 partials per batch and adds bo.

v2 dataflow (all matmuls bf16 with fp32 PSUM accum):
  - q/k/v are transposed AND cast to bf16 on the HOST -> qT/kT/vT
    [DIM, S] in HBM. No on-chip input transposes or casts; DMA traffic
    halves vs f32 naturals.
  - Weights pre-cast to bf16 on host (columns of Wq / rows of Wo
    permuted so local q-head h sits in tile h%4 at partition (h//4)*64,
    matching its kv head's partition base in kxT).
  - Projections: kxT/vxT first (phase A), then per 512-row i-block:
    qxT, attention, output projection. Biases added on DVE
    (tensor_scalar) during PSUM->SBUF eviction.
  - Scores: the two kv heads of a q-head pair run as row-tiled
    concurrent matmuls (K=64 each, PE row halves 0-63 / 64-127) into
    one 2-bank PSUM tile; ONE Exp activation covers both (3-D AP skips
    the causally-masked tail). Triangular mask applied multiplicatively
    post-exp on diagonal j-blocks only; j-blocks above the diagonal are
    skipped entirely.
  - AV: stationary is [v_head (64 cols) | ones (64 cols)], so PSUM rows
    64:127 accumulate the softmax denominator replicated 64-wide.
    Normalization = full-width DVE reciprocal + 2 multiplies (no DMA
    broadcast, no single-partition ops).
  - Output projection: fp32 partial written straight from a [128, 2048]
    SBUF staging tile, 1 MiB per DMA.
"""

import numpy as np
import ml_dtypes

import concourse.bass as bass
import concourse.mybir as mybir
from concourse import bacc
from concourse.tile import TileContext
from concourse.bass_utils import run_bass_kernel_spmd

F32 = mybir.dt.float32
BF16 = mybir.dt.bfloat16
AF = mybir.ActivationFunctionType
ALU = mybir.AluOpType

B, S, DIM = 2, 2048, 2048
HQ, HKV, HD = 32, 8, 64
GROUP = HQ // HKV              # 4
NCORES = 8
KVSH = 4                       # kv-blocks (shards) per batch
CQ = (HQ // KVSH) * HD         # 512 q-proj cols per core (8 heads)
CK = (HKV // KVSH) * HD        # 128 kv-proj cols per core (2 heads)
NDC = DIM // 128               # 16 contraction chunks
NSS = S // 512                 # 4 sequence chunks of 512
NJB = S // 128                 # 16 j-blocks of 128


def _ap3(sl, mid_stride, mid_n, last_n):
    """3-D AP over a 2-D tile slice: [partitions, mid_n x mid_stride,
    last_n] (element strides)."""
    return bass.AP(tensor=sl.tensor, offset=sl.offset,
                   ap=[list(sl.ap[0]), [mid_stride, mid_n], [1, last_n]])


def build_nc(mode="causal"):
    nc = bacc.Bacc("TRN2", target_bir_lowering=False)

    qT = nc.dram_tensor("qT", [DIM, S], BF16, kind="ExternalInput")
    kT = nc.dram_tensor("kT", [DIM, S], BF16, kind="ExternalInput")
    vT = nc.dram_tensor("vT", [DIM, S], BF16, kind="ExternalInput")
    wq = nc.dram_tensor("wq", [DIM, CQ], BF16, kind="ExternalInput")
    wk = nc.dram_tensor("wk", [DIM, CK], BF16, kind="ExternalInput")
    wv = nc.dram_tensor("wv", [DIM, CK], BF16, kind="ExternalInput")
    wo = nc.dram_tensor("wo", [CQ, DIM], BF16, kind="ExternalInput")
    bq = nc.dram_tensor("bq", [CQ], F32, kind="ExternalInput")
    bk = nc.dram_tensor("bk", [CK], F32, kind="ExternalInput")
    bv = nc.dram_tensor("bv", [CK], F32, kind="ExternalInput")
    tri2 = nc.dram_tensor("tri2", [128, 256], BF16, kind="ExternalInput")
    ident = nc.dram_tensor("ident", [128, 128], BF16, kind="ExternalInput")
    mbias = None
    if mode == "dense":
        mbias = nc.dram_tensor("mbias", [S, S], F32, kind="ExternalInput")
    out = nc.dram_tensor("out", [S, DIM], F32, kind="ExternalOutput")

    causal = mode == "causal"

    with TileContext(nc) as tc:
        with (
            tc.tile_pool(name="consts", bufs=1) as consts,
            tc.tile_pool(name="w", bufs=1) as wpool,
            tc.tile_pool(name="qt", bufs=1) as qtp,
            tc.tile_pool(name="stg", bufs=32) as stg,
            tc.tile_pool(name="acts", bufs=1) as acts,
            tc.tile_pool(name="vsb", bufs=2) as vxsb,
            tc.tile_pool(name="exp", bufs=3) as expp,
            tc.tile_pool(name="nm", bufs=2) as nmp,
            tc.tile_pool(name="ob", bufs=2) as obp,
            tc.tile_pool(name="mb", bufs=2) as mbp,
            tc.tile_pool(name="ps_sp", bufs=2, space="PSUM") as ps_sp,
            tc.tile_pool(name="ps_at", bufs=1, space="PSUM") as ps_at,
            tc.tile_pool(name="ps_pj", bufs=2, space="PSUM") as ps_pj,
        ):
            # ---- constants ----
            tri2_t = consts.tile([128, 256], BF16, tag="tri2")
            nc.sync.dma_start(out=tri2_t[:, :], in_=tri2[:, :])
            id_t = consts.tile([128, 128], BF16, tag="id")
            nc.sync.dma_start(out=id_t[:, :], in_=ident[:, :])
            bq_t = consts.tile([128, 4], F32, tag="bq")
            nc.sync.dma_start(
                out=bq_t[:, :],
                in_=bass.AP(tensor=bq[0:1].tensor, offset=0,
                            ap=[[1, 128], [128, 4]]))
            bk_t = consts.tile([128, 1], F32, tag="bk")
            nc.sync.dma_start(
                out=bk_t[:, :],
                in_=bass.AP(tensor=bk[0:1].tensor, offset=0,
                            ap=[[1, 128], [128, 1]]))
            bv_t = consts.tile([128, 1], F32, tag="bv")
            nc.sync.dma_start(
                out=bv_t[:, :],
                in_=bass.AP(tensor=bv[0:1].tensor, offset=0,
                            ap=[[1, 128], [128, 1]]))

            # ---- weights (already bf16) ----
            wq_t, wk_t, wv_t, wo_t = [], [], [], []
            for dc in range(NDC):
                t = wpool.tile([128, CQ], BF16, tag=f"wq{dc}")
                nc.sync.dma_start(out=t[:, :],
                                  in_=wq[dc * 128:(dc + 1) * 128, :])
                wq_t.append(t)
            for dc in range(NDC):
                t = wpool.tile([128, CK], BF16, tag=f"wk{dc}")
                nc.sync.dma_start(out=t[:, :],
                                  in_=wk[dc * 128:(dc + 1) * 128, :])
                wk_t.append(t)
                t = wpool.tile([128, CK], BF16, tag=f"wv{dc}")
                nc.sync.dma_start(out=t[:, :],
                                  in_=wv[dc * 128:(dc + 1) * 128, :])
                wv_t.append(t)
            for cc in range(4):
                t = wpool.tile([128, DIM], BF16, tag=f"wo{cc}")
                nc.sync.dma_start(out=t[:, :],
                                  in_=wo[cc * 128:(cc + 1) * 128, :])
                wo_t.append(t)

            # ---- qT loads: [128, 1024] per (dc, half); the second half
            # is emitted at its phase-B use point (slot grants follow
            # emission order, so an up-front emit would deadlock) ----
            qT_t = {}

            def load_qT(sh):
                for dc in range(NDC):
                    t = qtp.tile([128, 1024], BF16, tag=f"qT{dc}",
                                 name=f"qT{dc}_{sh}")
                    nc.gpsimd.dma_start(
                        out=t[:, :],
                        in_=qT[dc * 128:(dc + 1) * 128,
                               sh * 1024:(sh + 1) * 1024])
                    qT_t[(dc, sh)] = t

            load_qT(0)

            # ---- persistent activations ----
            qxT = {}   # (cc, ss) -> [128, 512]; rows 0:64 head cc (kv0),
            #            rows 64:128 head cc+4 (kv1)
            kxT = {}   # ss -> [128, 512]
            attnT = {}  # (pair, ss) -> [128, 512]
            for ss in range(NSS):
                kxT[ss] = acts.tile([128, 512], BF16, tag=f"kx{ss}", name=f"kx{ss}")
                for cc in range(4):
                    qxT[(cc, ss)] = acts.tile([128, 512], BF16,
                                              tag=f"qx{cc}_{ss}", name=f"qx{cc}_{ss}")
                    attnT[(cc, ss)] = acts.tile([128, 512], BF16,
                                                tag=f"at{cc}_{ss}", name=f"at{cc}_{ss}")
            # [v_head | 64 ones cols] per kv head, per j-block
            vx1r = []
            for jb in range(NJB):
                t = acts.tile([128, 256], BF16, tag=f"vp{jb}", name=f"vp{jb}")
                nc.vector.memset(t[:, 64:128], 1.0)
                nc.vector.memset(t[:, 192:256], 1.0)
                vx1r.append(t)

            # ---- phase A: k/v projections + v transpose ----
            for sh in range(2):
                ktl, vtl = {}, {}
                for dc in range(NDC):
                    t = stg.tile([128, 1024], BF16, tag="kv", name="kvstg")
                    nc.gpsimd.dma_start(
                        out=t[:, :],
                        in_=kT[dc * 128:(dc + 1) * 128,
                               sh * 1024:(sh + 1) * 1024])
                    ktl[dc] = t
                for dc in range(NDC):
                    t = stg.tile([128, 1024], BF16, tag="kv", name="kvstg")
                    nc.gpsimd.dma_start(
                        out=t[:, :],
                        in_=vT[dc * 128:(dc + 1) * 128,
                               sh * 1024:(sh + 1) * 1024])
                    vtl[dc] = t
                for ss in (2 * sh, 2 * sh + 1):
                    lo = (ss % 2) * 512
                    ps = ps_pj.tile([128, 512], F32, tag="pj")
                    for dc in range(NDC):
                        nc.tensor.matmul(ps[:, :], wk_t[dc][:, :],
                                         ktl[dc][:, lo:lo + 512],
                                         start=(dc == 0),
                                         stop=(dc == NDC - 1))
                    nc.vector.tensor_scalar_add(kxT[ss][:, :], ps[:, :],
                                                bk_t[:, 0:1])
                    ps = ps_pj.tile([128, 512], F32, tag="pj")
                    for dc in range(NDC):
                        nc.tensor.matmul(ps[:, :], wv_t[dc][:, :],
                                         vtl[dc][:, lo:lo + 512],
                                         start=(dc == 0),
                                         stop=(dc == NDC - 1))
                    vsb = vxsb.tile([128, 512], BF16, tag="vsb")
                    nc.vector.tensor_scalar_add(vsb[:, :], ps[:, :],
                                                bv_t[:, 0:1])
                    vtp = ps_pj.tile([128, 512], BF16, tag="pj")
                    for jr in range(4):
                        nc.tensor.transpose(vtp[:, jr * 128:(jr + 1) * 128],
                                            vsb[:, jr * 128:(jr + 1) * 128],
                                            id_t[:, :])
                    for jr in range(4):
                        jb = ss * 4 + jr
                        nc.vector.tensor_copy(
                            vx1r[jb][:, 0:64],
                            vtp[:, jr * 128:jr * 128 + 64])
                        nc.vector.tensor_copy(
                            vx1r[jb][:, 128:192],
                            vtp[:, jr * 128 + 64:jr * 128 + 128])

            # ---- phase B: per i-block ----
            for ss in range(NSS):
                s0 = ss * 512
                sh, lo = ss // 2, (ss % 2) * 512
                if ss == 2:
                    load_qT(1)
                # GEMM1 q: qxT for this i-block
                for cc in range(4):
                    ps = ps_pj.tile([128, 512], F32, tag="pj")
                    for dc in range(NDC):
                        nc.tensor.matmul(
                            ps[:, :], wq_t[dc][:, cc * 128:(cc + 1) * 128],
                            qT_t[(dc, sh)][:, lo:lo + 512],
                            start=(dc == 0), stop=(dc == NDC - 1))
                    nc.vector.tensor_scalar_add(qxT[(cc, ss)][:, :],
                                                ps[:, :], bq_t[:, cc:cc + 1])

                # attention: 4 head-pairs (cc, cc+4)
                njb = 4 * (ss + 1) if causal else NJB
                for pair in range(4):
                    qx = qxT[(pair, ss)]
                    at = ps_at.tile([128, 1024], F32, tag="at")
                    for jb in range(njb):
                        jss, jr = jb // 4, jb % 4
                        off = max(0, jb * 128 - s0) if causal else 0
                        N = 512 - off
                        sp = ps_sp.tile([128, 1024], F32, tag="sp")
                        nc.tensor.matmul(
                            sp[:, 0:N],
                            kxT[jss][0:64, jr * 128:(jr + 1) * 128],
                            qx[0:64, off:512], start=True, stop=True)
                        nc.tensor.matmul(
                            sp[:, 512:512 + N],
                            kxT[jss][64:128, jr * 128:(jr + 1) * 128],
                            qx[64:128, off:512], start=True, stop=True)
                        if mode == "dense":
                            mb = mbp.tile([128, 512], F32, tag="mb")
                            nc.sync.dma_start(
                                out=mb[:, 0:N],
                                in_=mbias[jb * 128:(jb + 1) * 128,
                                          s0 + off:s0 + 512])
                            nc.vector.tensor_tensor(
                                sp[:, 0:N], sp[:, 0:N], mb[:, 0:N], ALU.add)
                            nc.vector.tensor_tensor(
                                sp[:, 512:512 + N], sp[:, 512:512 + N],
                                mb[:, 0:N], ALU.add)
                        ex = expp.tile([128, 1024], BF16, tag="ex")
                        nc.scalar.activation(
                            _ap3(ex[:, 0:1024], 512, 2, N),
                            _ap3(sp[:, 0:1024], 512, 2, N),
                            AF.Exp, scale=0.125)
                        if causal and jss == ss:
                            nc.vector.tensor_tensor(
                                _ap3(ex[:, 0:1024], 512, 2, 128),
                                _ap3(ex[:, 0:1024], 512, 2, 128),
                                _ap3(tri2_t[:, 0:256], 128, 2, 128),
                                ALU.mult)
                        nc.tensor.matmul(
                            at[:, off:512], vx1r[jb][:, 0:128],
                            ex[:, 0:N],
                            start=(jb == 0), stop=(jb == njb - 1))
                        nc.tensor.matmul(
                            at[:, 512 + off:1024], vx1r[jb][:, 128:256],
                            ex[:, 512:512 + N],
                            start=(jb == 0), stop=(jb == njb - 1))
                    # normalize: rows 64:128 hold the denominator,
                    # replicated 64-wide by the ones columns
                    nm = nmp.tile([64, 1024], F32, tag="nm")
                    nc.vector.reciprocal(nm[:, :], at[64:128, 0:1024])
                    aT = attnT[(pair, ss)]
                    nc.vector.tensor_tensor(
                        aT[0:64, :], at[0:64, 0:512], nm[0:64, 0:512],
                        ALU.mult)
                    nc.vector.tensor_tensor(
                        aT[64:128, :], at[0:64, 512:1024],
                        nm[0:64, 512:1024], ALU.mult)

                # GEMM4: output projection (fp32 partial)
                for ic in range(4):
                    i0 = ic * 128
                    for hf in range(2):
                        ob = obp.tile([128, 1024], F32, tag="ob")
                        for e2 in range(2):
                            ec = hf * 2 + e2
                            g4 = ps_pj.tile([128, 512], F32, tag="pj")
                            for cc2 in range(4):
                                nc.tensor.matmul(
                                    g4[:, :],
                                    attnT[(cc2, ss)][:, i0:i0 + 128],
                                    wo_t[cc2][:, ec * 512:(ec + 1) * 512],
                                    start=(cc2 == 0), stop=(cc2 == 3))
                            nc.vector.tensor_copy(
                                ob[:, e2 * 512:(e2 + 1) * 512], g4[:, :])
                        nc.sync.dma_start(
                            out=out[s0 + i0:s0 + i0 + 128,
                                    hf * 1024:(hf + 1) * 1024],
                            in_=ob[:, :])
    nc.finalize()
    return nc


_CACHE = {}


def _get_nc(mode):
    if mode not in _CACHE:
        _CACHE[mode] = build_nc(mode)
    return _CACHE[mode]


def kernel(q, k, v, mask, Wq, bq, Wk, bk, Wv, bv, Wo, bo):
    bf = ml_dtypes.bfloat16
    q = np.asarray(q, np.float32)
    k = np.asarray(k, np.float32)
    v = np.asarray(v, np.float32)
    mask = np.asarray(mask)
    Wq = np.asarray(Wq, np.float32)
    Wk = np.asarray(Wk, np.float32)
    Wv = np.asarray(Wv, np.float32)
    Wo = np.asarray(Wo, np.float32)
    bq = np.asarray(bq, np.float32)
    bk = np.asarray(bk, np.float32)
    bv = np.asarray(bv, np.float32)
    bo = np.asarray(bo, np.float32)

    m = mask.astype(np.float64)
    if np.array_equal(m, np.tril(np.ones((S, S)))):
        mode = "causal"
    elif np.all(m == 1):
        mode = "none"
    else:
        mode = "dense"

    nc = _get_nc(mode)
    tri = np.triu(np.ones((128, 128), np.float32))
    tri2_np = np.concatenate([tri, tri], axis=1).astype(bf)
    id_np = np.eye(128).astype(bf)

    # On-chip layout places local q head h in tile h%4 at partition
    # (h//4)*64 so q/k partition bases match in the scores matmul. Permute
    # Wq columns / Wo rows / bq accordingly: tile cc holds heads (cc, cc+4).
    head_perm = [h for cc in range(4) for h in (cc, cc + 4)]
    col_perm = np.concatenate(
        [np.arange(h * HD, (h + 1) * HD) for h in head_perm])

    # per-batch transposed bf16 inputs (shared across the 4 kv shards)
    qT_b = [np.ascontiguousarray(q[b].astype(bf).T) for b in range(B)]
    kT_b = [np.ascontiguousarray(k[b].astype(bf).T) for b in range(B)]
    vT_b = [np.ascontiguousarray(v[b].astype(bf).T) for b in range(B)]

    in_maps = []
    for core in range(NCORES):
        b, kb = core // KVSH, core % KVSH
        wq_sh = Wq[:, kb * CQ:(kb + 1) * CQ][:, col_perm]
        wo_sh = Wo[kb * CQ:(kb + 1) * CQ, :][col_perm, :]
        bq_sh = bq[kb * CQ:(kb + 1) * CQ][col_perm]
        im = {
            "qT": qT_b[b],
            "kT": kT_b[b],
            "vT": vT_b[b],
            "wq": np.ascontiguousarray(wq_sh.astype(bf)),
            "wk": np.ascontiguousarray(
                Wk[:, kb * CK:(kb + 1) * CK].astype(bf)),
            "wv": np.ascontiguousarray(
                Wv[:, kb * CK:(kb + 1) * CK].astype(bf)),
            "wo": np.ascontiguousarray(wo_sh.astype(bf)),
            "bq": np.ascontiguousarray(bq_sh),
            "bk": np.ascontiguousarray(bk[kb * CK:(kb + 1) * CK]),
            "bv": np.ascontiguousarray(bv[kb * CK:(kb + 1) * CK]),
            "tri2": tri2_np,
            "ident": id_np,
        }
        if mode == "dense":
            with np.errstate(divide="ignore"):
                bias = -(1.0 / mask.astype(np.float32) + 1.0)
            im["mbias"] = np.ascontiguousarray(bias.T * 8.0)
        in_maps.append(im)

    res = run_bass_kernel_spmd(nc, in_maps, core_ids=list(range(NCORES)))
    outs = [r["out"] for r in res.results]
    full = np.empty((B, S, DIM), np.float32)
    for b in range(B):
        acc = outs[b * KVSH].astype(np.float32)
        for kb in range(1, KVSH):
            acc = acc + outs[b * KVSH + kb]
        full[b] = acc + bo[None, :]
    return full


# revision 7
# speedup vs baseline: 1.5642x; 1.0552x over previous
"""Grouped-query attention (GQA) Trainium2 Bass kernel, v2.

Problem: B=2, S=2048, DIM=2048, HQ=32, HKV=8, HEAD_DIM=64, causal mask.
Sharding: 8 cores = 2 (batch) x 4 (kv-head groups). Core c handles batch
c//4 and kv-block c%4 (2 kv heads, 8 q heads). Wq/Wk/Wv sharded
column-wise, Wo row-wise; each core writes a partial [S, DIM] output;
host sums the 4 partials per batch and adds bo.

v2 dataflow (all matmuls bf16 with fp32 PSUM accum):
  - q/k/v are transposed AND cast to bf16 on the HOST -> qT/kT/vT
    [DIM, S] in HBM. No on-chip input transposes or casts; DMA traffic
    halves vs f32 naturals.
  - Weights pre-cast to bf16 on host (columns of Wq / rows of Wo
    permuted so local q-head h sits in tile h%4 at partition (h//4)*64,
    matching its kv head's partition base in kxT).
  - Projections: kxT/vxT first (phase A), then per 512-row i-block:
    qxT, attention, output projection. Biases added on DVE
    (tensor_scalar) during PSUM->SBUF eviction.
  - Scores: the two kv heads of a q-head pair run as row-tiled
    concurrent matmuls (K=64 each, PE row halves 0-63 / 64-127) into
    one 2-bank PSUM tile; ONE Exp activation covers both (3-D AP skips
    the causally-masked tail). Triangular mask applied multiplicatively
    post-exp on diagonal j-blocks only; j-blocks above the diagonal are
    skipped entirely.
  - AV: stationary is [v_head (64 cols) | ones (64 cols)], so PSUM rows
    64:127 accumulate the softmax denominator replicated 64-wide.
    Normalization = full-width DVE reciprocal + 2 multiplies (no DMA
    broadcast, no single-partition ops).
  - Output projection: fp32 partial written straight from a [128, 2048]
    SBUF staging tile, 1 MiB per DMA.
"""

import numpy as np
import ml_dtypes

import concourse.bass as bass
import concourse.mybir as mybir
from concourse import bacc
from concourse.tile import TileContext
from concourse.bass_utils import run_bass_kernel_spmd

F32 = mybir.dt.float32
BF16 = mybir.dt.bfloat16
AF = mybir.ActivationFunctionType
ALU = mybir.AluOpType

B, S, DIM = 2, 2048, 2048
HQ, HKV, HD = 32, 8, 64
GROUP = HQ // HKV              # 4
NCORES = 8
KVSH = 4                       # kv-blocks (shards) per batch
CQ = (HQ // KVSH) * HD         # 512 q-proj cols per core (8 heads)
CK = (HKV // KVSH) * HD        # 128 kv-proj cols per core (2 heads)
NDC = DIM // 128               # 16 contraction chunks
NSS = S // 512                 # 4 sequence chunks of 512
NJB = S // 128                 # 16 j-blocks of 128


def _ap3(sl, mid_stride, mid_n, last_n):
    """3-D AP over a 2-D tile slice: [partitions, mid_n x mid_stride,
    last_n] (element strides)."""
    return bass.AP(tensor=sl.tensor, offset=sl.offset,
                   ap=[list(sl.ap[0]), [mid_stride, mid_n], [1, last_n]])


def build_nc(mode="causal"):
    nc = bacc.Bacc("TRN2", target_bir_lowering=False)

    qT = nc.dram_tensor("qT", [DIM, S], BF16, kind="ExternalInput")
    kT = nc.dram_tensor("kT", [DIM, S], BF16, kind="ExternalInput")
    vT = nc.dram_tensor("vT", [DIM, S], BF16, kind="ExternalInput")
    wq = nc.dram_tensor("wq", [DIM, CQ], BF16, kind="ExternalInput")
    wk = nc.dram_tensor("wk", [DIM, CK], BF16, kind="ExternalInput")
    wv = nc.dram_tensor("wv", [DIM, CK], BF16, kind="ExternalInput")
    wo = nc.dram_tensor("wo", [CQ, DIM], BF16, kind="ExternalInput")
    bq = nc.dram_tensor("bq", [CQ], F32, kind="ExternalInput")
    bk = nc.dram_tensor("bk", [CK], F32, kind="ExternalInput")
    bv = nc.dram_tensor("bv", [CK], F32, kind="ExternalInput")
    tri2 = nc.dram_tensor("tri2", [128, 256], BF16, kind="ExternalInput")
    ident = nc.dram_tensor("ident", [128, 128], BF16, kind="ExternalInput")
    mbias = None
    if mode == "dense":
        mbias = nc.dram_tensor("mbias", [S, S], F32, kind="ExternalInput")
    out = nc.dram_tensor("out", [S, DIM], F32, kind="ExternalOutput")

    causal = mode == "causal"

    with TileContext(nc) as tc:
        with (
            tc.tile_pool(name="consts", bufs=1) as consts,
            tc.tile_pool(name="w", bufs=1) as wpool,
            tc.tile_pool(name="qt", bufs=1) as qtp,
            tc.tile_pool(name="stg", bufs=32) as stg,
            tc.tile_pool(name="acts", bufs=1) as acts,
            tc.tile_pool(name="vsb", bufs=2) as vxsb,
            tc.tile_pool(name="exp", bufs=3) as expp,
            tc.tile_pool(name="nm", bufs=2) as nmp,
            tc.tile_pool(name="ob", bufs=2) as obp,
            tc.tile_pool(name="mb", bufs=2) as mbp,
            tc.tile_pool(name="ps_sp", bufs=2, space="PSUM") as ps_sp,
            tc.tile_pool(name="ps_at", bufs=1, space="PSUM") as ps_at,
            tc.tile_pool(name="ps_pj", bufs=2, space="PSUM") as ps_pj,
        ):
            # ---- constants ----
            tri2_t = consts.tile([128, 256], BF16, tag="tri2")
            nc.sync.dma_start(out=tri2_t[:, :], in_=tri2[:, :])
            id_t = consts.tile([128, 128], BF16, tag="id")
            nc.sync.dma_start(out=id_t[:, :], in_=ident[:, :])
            bq_t = consts.tile([128, 4], F32, tag="bq")
            nc.sync.dma_start(
                out=bq_t[:, :],
                in_=bass.AP(tensor=bq[0:1].tensor, offset=0,
                            ap=[[1, 128], [128, 4]]))
            bk_t = consts.tile([128, 1], F32, tag="bk")
            nc.sync.dma_start(
                out=bk_t[:, :],
                in_=bass.AP(tensor=bk[0:1].tensor, offset=0,
                            ap=[[1, 128], [128, 1]]))
            bv_t = consts.tile([128, 1], F32, tag="bv")
            nc.sync.dma_start(
                out=bv_t[:, :],
                in_=bass.AP(tensor=bv[0:1].tensor, offset=0,
                            ap=[[1, 128], [128, 1]]))

            # ---- weights (already bf16) ----
            wq_t, wk_t, wv_t, wo_t = [], [], [], []
            for dc in range(NDC):
                t = wpool.tile([128, CK], BF16, tag=f"wk{dc}")
                nc.sync.dma_start(out=t[:, :],
                                  in_=wk[dc * 128:(dc + 1) * 128, :])
                wk_t.append(t)
                t = wpool.tile([128, CK], BF16, tag=f"wv{dc}")
                nc.sync.dma_start(out=t[:, :],
                                  in_=wv[dc * 128:(dc + 1) * 128, :])
                wv_t.append(t)
            for dc in range(NDC):
                t = wpool.tile([128, CQ], BF16, tag=f"wq{dc}")
                nc.sync.dma_start(out=t[:, :],
                                  in_=wq[dc * 128:(dc + 1) * 128, :])
                wq_t.append(t)
            for cc in range(4):
                t = wpool.tile([128, DIM], BF16, tag=f"wo{cc}")
                nc.sync.dma_start(out=t[:, :],
                                  in_=wo[cc * 128:(cc + 1) * 128, :])
                wo_t.append(t)

            # ---- qT loads: [128, 1024] per (dc, half); the second half
            # is emitted at its phase-B use point (slot grants follow
            # emission order, so an up-front emit would deadlock) ----
            qT_t = {}

            def load_qT(sh):
                for dc in range(NDC):
                    t = qtp.tile([128, 1024], BF16, tag=f"qT{dc}",
                                 name=f"qT{dc}_{sh}")
                    nc.gpsimd.dma_start(
                        out=t[:, :],
                        in_=qT[dc * 128:(dc + 1) * 128,
                               sh * 1024:(sh + 1) * 1024])
                    qT_t[(dc, sh)] = t

            load_qT(0)

            # ---- persistent activations ----
            qxT = {}   # (cc, ss) -> [128, 512]; rows 0:64 head cc (kv0),
            #            rows 64:128 head cc+4 (kv1)
            kxT = {}   # ss -> [128, 512]
            attnT = {}  # (pair, ss) -> [128, 512]
            for ss in range(NSS):
                kxT[ss] = acts.tile([128, 512], BF16, tag=f"kx{ss}", name=f"kx{ss}")
                for cc in range(4):
                    qxT[(cc, ss)] = acts.tile([128, 512], BF16,
                                              tag=f"qx{cc}_{ss}", name=f"qx{cc}_{ss}")
                    attnT[(cc, ss)] = acts.tile([128, 512], BF16,
                                                tag=f"at{cc}_{ss}", name=f"at{cc}_{ss}")
            # [v_head | 64 ones cols] per kv head, per j-block
            vx1r = []
            for jb in range(NJB):
                t = acts.tile([128, 256], BF16, tag=f"vp{jb}", name=f"vp{jb}")
                nc.vector.memset(t[:, 64:128], 1.0)
                nc.vector.memset(t[:, 192:256], 1.0)
                vx1r.append(t)

            # ---- phase A: k/v projections + v transpose ----
            for sh in range(2):
                ktl, vtl = {}, {}
                for dc in range(NDC):
                    t = stg.tile([128, 1024], BF16, tag="kv", name="kvstg")
                    nc.gpsimd.dma_start(
                        out=t[:, :],
                        in_=kT[dc * 128:(dc + 1) * 128,
                               sh * 1024:(sh + 1) * 1024])
                    ktl[dc] = t
                for dc in range(NDC):
                    t = stg.tile([128, 1024], BF16, tag="kv", name="kvstg")
                    nc.gpsimd.dma_start(
                        out=t[:, :],
                        in_=vT[dc * 128:(dc + 1) * 128,
                               sh * 1024:(sh + 1) * 1024])
                    vtl[dc] = t
                for ss in (2 * sh, 2 * sh + 1):
                    lo = (ss % 2) * 512
                    ps = ps_pj.tile([128, 512], F32, tag="pj")
                    for dc in range(NDC):
                        nc.tensor.matmul(ps[:, :], wk_t[dc][:, :],
                                         ktl[dc][:, lo:lo + 512],
                                         start=(dc == 0),
                                         stop=(dc == NDC - 1))
                    nc.vector.tensor_scalar_add(kxT[ss][:, :], ps[:, :],
                                                bk_t[:, 0:1])
                    ps = ps_pj.tile([128, 512], F32, tag="pj")
                    for dc in range(NDC):
                        nc.tensor.matmul(ps[:, :], wv_t[dc][:, :],
                                         vtl[dc][:, lo:lo + 512],
                                         start=(dc == 0),
                                         stop=(dc == NDC - 1))
                    vsb = vxsb.tile([128, 512], BF16, tag="vsb")
                    nc.vector.tensor_scalar_add(vsb[:, :], ps[:, :],
                                                bv_t[:, 0:1])
                    vtp = ps_pj.tile([128, 512], BF16, tag="pj")
                    for jr in range(4):
                        nc.tensor.transpose(vtp[:, jr * 128:(jr + 1) * 128],
                                            vsb[:, jr * 128:(jr + 1) * 128],
                                            id_t[:, :])
                    for jr in range(4):
                        jb = ss * 4 + jr
                        nc.vector.tensor_copy(
                            vx1r[jb][:, 0:64],
                            vtp[:, jr * 128:jr * 128 + 64])
                        nc.vector.tensor_copy(
                            vx1r[jb][:, 128:192],
                            vtp[:, jr * 128 + 64:jr * 128 + 128])

            # ---- phase B: per i-block ----
            for ss in range(NSS):
                s0 = ss * 512
                sh, lo = ss // 2, (ss % 2) * 512
                if ss == 2:
                    load_qT(1)
                # GEMM1 q: qxT for this i-block
                for cc in range(4):
                    ps = ps_pj.tile([128, 512], F32, tag="pj")
                    for dc in range(NDC):
                        nc.tensor.matmul(
                            ps[:, :], wq_t[dc][:, cc * 128:(cc + 1) * 128],
                            qT_t[(dc, sh)][:, lo:lo + 512],
                            start=(dc == 0), stop=(dc == NDC - 1))
                    nc.vector.tensor_scalar_add(qxT[(cc, ss)][:, :],
                                                ps[:, :], bq_t[:, cc:cc + 1])

                # attention: 4 head-pairs (cc, cc+4)
                njb = 4 * (ss + 1) if causal else NJB
                for pair in range(4):
                    qx = qxT[(pair, ss)]
                    at = ps_at.tile([128, 1024], F32, tag="at")
                    for jb in range(njb):
                        jss, jr = jb // 4, jb % 4
                        off = max(0, jb * 128 - s0) if causal else 0
                        N = 512 - off
                        sp = ps_sp.tile([128, 1024], F32, tag="sp")
                        nc.tensor.matmul(
                            sp[:, 0:N],
                            kxT[jss][0:64, jr * 128:(jr + 1) * 128],
                            qx[0:64, off:512], start=True, stop=True)
                        nc.tensor.matmul(
                            sp[:, 512:512 + N],
                            kxT[jss][64:128, jr * 128:(jr + 1) * 128],
                            qx[64:128, off:512], start=True, stop=True)
                        if mode == "dense":
                            mb = mbp.tile([128, 512], F32, tag="mb")
                            nc.sync.dma_start(
                                out=mb[:, 0:N],
                                in_=mbias[jb * 128:(jb + 1) * 128,
                                          s0 + off:s0 + 512])
                            nc.vector.tensor_tensor(
                                sp[:, 0:N], sp[:, 0:N], mb[:, 0:N], ALU.add)
                            nc.vector.tensor_tensor(
                                sp[:, 512:512 + N], sp[:, 512:512 + N],
                                mb[:, 0:N], ALU.add)
                        ex = expp.tile([128, 1024], BF16, tag="ex")
                        nc.scalar.activation(
                            _ap3(ex[:, 0:1024], 512, 2, N),
                            _ap3(sp[:, 0:1024], 512, 2, N),
                            AF.Exp, scale=0.125)
                        if causal and jss == ss:
                            nc.vector.tensor_tensor(
                                _ap3(ex[:, 0:1024], 512, 2, 128),
                                _ap3(ex[:, 0:1024], 512, 2, 128),
                                _ap3(tri2_t[:, 0:256], 128, 2, 128),
                                ALU.mult)
                        nc.tensor.matmul(
                            at[:, off:512], vx1r[jb][:, 0:128],
                            ex[:, 0:N],
                            start=(jb == 0), stop=(jb == njb - 1))
                        nc.tensor.matmul(
                            at[:, 512 + off:1024], vx1r[jb][:, 128:256],
                            ex[:, 512:512 + N],
                            start=(jb == 0), stop=(jb == njb - 1))
                    # normalize: rows 64:128 hold the denominator,
                    # replicated 64-wide by the ones columns
                    # 1/D = exp(-ln(D)) on ACT: Exp and Ln share one
                    # table set, and each call is ~6x cheaper than DVE
                    # InstReciprocal
                    lnD = nmp.tile([64, 1024], F32, tag="nm")
                    nc.scalar.activation(lnD[:, :], at[64:128, 0:1024], AF.Ln)
                    nm = nmp.tile([64, 1024], F32, tag="nm")
                    nc.scalar.activation(nm[:, :], lnD[:, :], AF.Exp,
                                         scale=-1.0)
                    aT = attnT[(pair, ss)]
                    nc.vector.tensor_tensor(
                        aT[0:64, :], at[0:64, 0:512], nm[0:64, 0:512],
                        ALU.mult)
                    nc.vector.tensor_tensor(
                        aT[64:128, :], at[0:64, 512:1024],
                        nm[0:64, 512:1024], ALU.mult)

                # GEMM4: output projection (fp32 partial)
                for ic in range(4):
                    i0 = ic * 128
                    for hf in range(2):
                        ob = obp.tile([128, 1024], F32, tag="ob")
                        for e2 in range(2):
                            ec = hf * 2 + e2
                            g4 = ps_pj.tile([128, 512], F32, tag="pj")
                            for cc2 in range(4):
                                nc.tensor.matmul(
                                    g4[:, :],
                                    attnT[(cc2, ss)][:, i0:i0 + 128],
                                    wo_t[cc2][:, ec * 512:(ec + 1) * 512],
                                    start=(cc2 == 0), stop=(cc2 == 3))
                            nc.vector.tensor_copy(
                                ob[:, e2 * 512:(e2 + 1) * 512], g4[:, :])
                        nc.sync.dma_start(
                            out=out[s0 + i0:s0 + i0 + 128,
                                    hf * 1024:(hf + 1) * 1024],
                            in_=ob[:, :])
    nc.finalize()
    return nc


_CACHE = {}


def _get_nc(mode):
    if mode not in _CACHE:
        _CACHE[mode] = build_nc(mode)
    return _CACHE[mode]


def kernel(q, k, v, mask, Wq, bq, Wk, bk, Wv, bv, Wo, bo):
    bf = ml_dtypes.bfloat16
    q = np.asarray(q, np.float32)
    k = np.asarray(k, np.float32)
    v = np.asarray(v, np.float32)
    mask = np.asarray(mask)
    Wq = np.asarray(Wq, np.float32)
    Wk = np.asarray(Wk, np.float32)
    Wv = np.asarray(Wv, np.float32)
    Wo = np.asarray(Wo, np.float32)
    bq = np.asarray(bq, np.float32)
    bk = np.asarray(bk, np.float32)
    bv = np.asarray(bv, np.float32)
    bo = np.asarray(bo, np.float32)

    m = mask.astype(np.float64)
    if np.array_equal(m, np.tril(np.ones((S, S)))):
        mode = "causal"
    elif np.all(m == 1):
        mode = "none"
    else:
        mode = "dense"

    nc = _get_nc(mode)
    tri = np.triu(np.ones((128, 128), np.float32))
    tri2_np = np.concatenate([tri, tri], axis=1).astype(bf)
    id_np = np.eye(128).astype(bf)

    # On-chip layout places local q head h in tile h%4 at partition
    # (h//4)*64 so q/k partition bases match in the scores matmul. Permute
    # Wq columns / Wo rows / bq accordingly: tile cc holds heads (cc, cc+4).
    head_perm = [h for cc in range(4) for h in (cc, cc + 4)]
    col_perm = np.concatenate(
        [np.arange(h * HD, (h + 1) * HD) for h in head_perm])

    # per-batch transposed bf16 inputs (shared across the 4 kv shards)
    qT_b = [np.ascontiguousarray(q[b].astype(bf).T) for b in range(B)]
    kT_b = [np.ascontiguousarray(k[b].astype(bf).T) for b in range(B)]
    vT_b = [np.ascontiguousarray(v[b].astype(bf).T) for b in range(B)]

    in_maps = []
    for core in range(NCORES):
        b, kb = core // KVSH, core % KVSH
        wq_sh = Wq[:, kb * CQ:(kb + 1) * CQ][:, col_perm]
        wo_sh = Wo[kb * CQ:(kb + 1) * CQ, :][col_perm, :]
        bq_sh = bq[kb * CQ:(kb + 1) * CQ][col_perm]
        im = {
            "qT": qT_b[b],
            "kT": kT_b[b],
            "vT": vT_b[b],
            "wq": np.ascontiguousarray(wq_sh.astype(bf)),
            "wk": np.ascontiguousarray(
                Wk[:, kb * CK:(kb + 1) * CK].astype(bf)),
            "wv": np.ascontiguousarray(
                Wv[:, kb * CK:(kb + 1) * CK].astype(bf)),
            "wo": np.ascontiguousarray(wo_sh.astype(bf)),
            "bq": np.ascontiguousarray(bq_sh),
            "bk": np.ascontiguousarray(bk[kb * CK:(kb + 1) * CK]),
            "bv": np.ascontiguousarray(bv[kb * CK:(kb + 1) * CK]),
            "tri2": tri2_np,
            "ident": id_np,
        }
        if mode == "dense":
            with np.errstate(divide="ignore"):
                bias = -(1.0 / mask.astype(np.float32) + 1.0)
            im["mbias"] = np.ascontiguousarray(bias.T * 8.0)
        in_maps.append(im)

    res = run_bass_kernel_spmd(nc, in_maps, core_ids=list(range(NCORES)))
    outs = [r["out"] for r in res.results]
    full = np.empty((B, S, DIM), np.float32)
    for b in range(B):
        acc = outs[b * KVSH].astype(np.float32)
        for kb in range(1, KVSH):
            acc = acc + outs[b * KVSH + kb]
        full[b] = acc + bo[None, :]
    return full


# revision 8
# speedup vs baseline: 1.6845x; 1.0769x over previous
"""Grouped-query attention (GQA) Trainium2 Bass kernel, v2.

Problem: B=2, S=2048, DIM=2048, HQ=32, HKV=8, HEAD_DIM=64, causal mask.
Sharding: 8 cores = 2 (batch) x 4 (kv-head groups). Core c handles batch
c//4 and kv-block c%4 (2 kv heads, 8 q heads). Wq/Wk/Wv sharded
column-wise, Wo row-wise; each core writes a partial [S, DIM] output;
host sums the 4 partials per batch and adds bo.

v2 dataflow (all matmuls bf16 with fp32 PSUM accum):
  - q/k/v are transposed AND cast to bf16 on the HOST -> qT/kT/vT
    [DIM, S] in HBM. No on-chip input transposes or casts; DMA traffic
    halves vs f32 naturals.
  - Weights pre-cast to bf16 on host (columns of Wq / rows of Wo
    permuted so local q-head h sits in tile h%4 at partition (h//4)*64,
    matching its kv head's partition base in kxT).
  - Projections: kxT/vxT first (phase A), then per 512-row i-block:
    qxT, attention, output projection. Biases added on DVE
    (tensor_scalar) during PSUM->SBUF eviction.
  - Scores: the two kv heads of a q-head pair run as row-tiled
    concurrent matmuls (K=64 each, PE row halves 0-63 / 64-127) into
    one 2-bank PSUM tile; ONE Exp activation covers both (3-D AP skips
    the causally-masked tail). Triangular mask applied multiplicatively
    post-exp on diagonal j-blocks only; j-blocks above the diagonal are
    skipped entirely.
  - AV: stationary is [v_head (64 cols) | ones (64 cols)], so PSUM rows
    64:127 accumulate the softmax denominator replicated 64-wide.
    Normalization = full-width DVE reciprocal + 2 multiplies (no DMA
    broadcast, no single-partition ops).
  - Output projection: fp32 partial written straight from a [128, 2048]
    SBUF staging tile, 1 MiB per DMA.
"""

import numpy as np
import ml_dtypes

import concourse.bass as bass
import concourse.mybir as mybir
from concourse import bacc
from concourse.tile import TileContext
from concourse.bass_utils import run_bass_kernel_spmd

# This kernel uses Exp (softmax) and Ln (denominator reciprocal via
# exp(-ln(D))) on the ACT engine. The table-load placement pass maps each
# function to the first table set containing it, which picks
# `exp_and_others` for Exp and `natural_log` for Ln and thrashes
# ACT_TABLE_LOADs (~1.3us + drain each) on every normalization. Both live
# in `natural_log_exp_and_others`; narrow the claimed contents of the
# other sets (names and dict order - hence set ids - are unchanged) so
# the pass settles on the shared set once.
_orig_get_act_tables = bacc.get_activation_tables


def _pinned_act_tables(arch):
    tabs = _orig_get_act_tables(arch)
    exp = mybir.ActivationFunctionType.Exp
    ln = mybir.ActivationFunctionType.Ln
    shared = "natural_log_exp_and_others"
    if shared in tabs and exp in tabs[shared] and ln in tabs[shared]:
        for name, funcs in tabs.items():
            if name != shared:
                tabs[name] = funcs - {exp, ln}
    return tabs


bacc.get_activation_tables = _pinned_act_tables

F32 = mybir.dt.float32
BF16 = mybir.dt.bfloat16
AF = mybir.ActivationFunctionType
ALU = mybir.AluOpType

B, S, DIM = 2, 2048, 2048
HQ, HKV, HD = 32, 8, 64
GROUP = HQ // HKV              # 4
NCORES = 8
KVSH = 4                       # kv-blocks (shards) per batch
CQ = (HQ // KVSH) * HD         # 512 q-proj cols per core (8 heads)
CK = (HKV // KVSH) * HD        # 128 kv-proj cols per core (2 heads)
NDC = DIM // 128               # 16 contraction chunks
NSS = S // 512                 # 4 sequence chunks of 512
NJB = S // 128                 # 16 j-blocks of 128


def _ap3(sl, mid_stride, mid_n, last_n):
    """3-D AP over a 2-D tile slice: [partitions, mid_n x mid_stride,
    last_n] (element strides)."""
    return bass.AP(tensor=sl.tensor, offset=sl.offset,
                   ap=[list(sl.ap[0]), [mid_stride, mid_n], [1, last_n]])


def build_nc(mode="causal"):
    nc = bacc.Bacc("TRN2", target_bir_lowering=False)

    qT = nc.dram_tensor("qT", [DIM, S], BF16, kind="ExternalInput")
    kT = nc.dram_tensor("kT", [DIM, S], BF16, kind="ExternalInput")
    vT = nc.dram_tensor("vT", [DIM, S], BF16, kind="ExternalInput")
    wq = nc.dram_tensor("wq", [DIM, CQ], BF16, kind="ExternalInput")
    wk = nc.dram_tensor("wk", [DIM, CK], BF16, kind="ExternalInput")
    wv = nc.dram_tensor("wv", [DIM, CK], BF16, kind="ExternalInput")
    wo = nc.dram_tensor("wo", [CQ, DIM], BF16, kind="ExternalInput")
    bq = nc.dram_tensor("bq", [CQ], F32, kind="ExternalInput")
    bk = nc.dram_tensor("bk", [CK], F32, kind="ExternalInput")
    bv = nc.dram_tensor("bv", [CK], F32, kind="ExternalInput")
    tri2 = nc.dram_tensor("tri2", [128, 256], BF16, kind="ExternalInput")
    ident = nc.dram_tensor("ident", [128, 128], BF16, kind="ExternalInput")
    mbias = None
    if mode == "dense":
        mbias = nc.dram_tensor("mbias", [S, S], F32, kind="ExternalInput")
    out = nc.dram_tensor("out", [S, DIM], F32, kind="ExternalOutput")

    causal = mode == "causal"

    with TileContext(nc) as tc:
        with (
            tc.tile_pool(name="consts", bufs=1) as consts,
            tc.tile_pool(name="w", bufs=1) as wpool,
            tc.tile_pool(name="qt", bufs=1) as qtp,
            tc.tile_pool(name="stg", bufs=32) as stg,
            tc.tile_pool(name="acts", bufs=1) as acts,
            tc.tile_pool(name="vsb", bufs=2) as vxsb,
            tc.tile_pool(name="exp", bufs=3) as expp,
            tc.tile_pool(name="nm", bufs=2) as nmp,
            tc.tile_pool(name="ob", bufs=2) as obp,
            tc.tile_pool(name="mb", bufs=2) as mbp,
            tc.tile_pool(name="ps_sp", bufs=2, space="PSUM") as ps_sp,
            tc.tile_pool(name="ps_at", bufs=1, space="PSUM") as ps_at,
            tc.tile_pool(name="ps_pj", bufs=2, space="PSUM") as ps_pj,
        ):
            # ---- constants ----
            tri2_t = consts.tile([128, 256], BF16, tag="tri2")
            nc.sync.dma_start(out=tri2_t[:, :], in_=tri2[:, :])
            id_t = consts.tile([128, 128], BF16, tag="id")
            nc.sync.dma_start(out=id_t[:, :], in_=ident[:, :])
            bq_t = consts.tile([128, 4], F32, tag="bq")
            nc.sync.dma_start(
                out=bq_t[:, :],
                in_=bass.AP(tensor=bq[0:1].tensor, offset=0,
                            ap=[[1, 128], [128, 4]]))
            bk_t = consts.tile([128, 1], F32, tag="bk")
            nc.sync.dma_start(
                out=bk_t[:, :],
                in_=bass.AP(tensor=bk[0:1].tensor, offset=0,
                            ap=[[1, 128], [128, 1]]))
            bv_t = consts.tile([128, 1], F32, tag="bv")
            nc.sync.dma_start(
                out=bv_t[:, :],
                in_=bass.AP(tensor=bv[0:1].tensor, offset=0,
                            ap=[[1, 128], [128, 1]]))

            # ---- weights (already bf16) ----
            wq_t, wk_t, wv_t, wo_t = [], [], [], []
            for dc in range(NDC):
                t = wpool.tile([128, CK], BF16, tag=f"wk{dc}")
                nc.sync.dma_start(out=t[:, :],
                                  in_=wk[dc * 128:(dc + 1) * 128, :])
                wk_t.append(t)
                t = wpool.tile([128, CK], BF16, tag=f"wv{dc}")
                nc.sync.dma_start(out=t[:, :],
                                  in_=wv[dc * 128:(dc + 1) * 128, :])
                wv_t.append(t)
            for dc in range(NDC):
                t = wpool.tile([128, CQ], BF16, tag=f"wq{dc}")
                nc.sync.dma_start(out=t[:, :],
                                  in_=wq[dc * 128:(dc + 1) * 128, :])
                wq_t.append(t)
            for cc in range(4):
                t = wpool.tile([128, DIM], BF16, tag=f"wo{cc}")
                nc.sync.dma_start(out=t[:, :],
                                  in_=wo[cc * 128:(cc + 1) * 128, :])
                wo_t.append(t)

            # ---- qT loads: [128, 1024] per (dc, half); the second half
            # is emitted at its phase-B use point (slot grants follow
            # emission order, so an up-front emit would deadlock) ----
            qT_t = {}

            def load_qT(sh):
                for dc in range(NDC):
                    t = qtp.tile([128, 1024], BF16, tag=f"qT{dc}",
                                 name=f"qT{dc}_{sh}")
                    nc.gpsimd.dma_start(
                        out=t[:, :],
                        in_=qT[dc * 128:(dc + 1) * 128,
                               sh * 1024:(sh + 1) * 1024])
                    qT_t[(dc, sh)] = t

            load_qT(0)

            # ---- persistent activations ----
            qxT = {}   # (cc, ss) -> [128, 512]; rows 0:64 head cc (kv0),
            #            rows 64:128 head cc+4 (kv1)
            kxT = {}   # ss -> [128, 512]
            attnT = {}  # (pair, ss) -> [128, 512]
            for ss in range(NSS):
                kxT[ss] = acts.tile([128, 512], BF16, tag=f"kx{ss}", name=f"kx{ss}")
                for cc in range(4):
                    qxT[(cc, ss)] = acts.tile([128, 512], BF16,
                                              tag=f"qx{cc}_{ss}", name=f"qx{cc}_{ss}")
                    attnT[(cc, ss)] = acts.tile([128, 512], BF16,
                                                tag=f"at{cc}_{ss}", name=f"at{cc}_{ss}")
            # [v_head | 64 ones cols] per kv head, per j-block
            vx1r = []
            for jb in range(NJB):
                t = acts.tile([128, 256], BF16, tag=f"vp{jb}", name=f"vp{jb}")
                nc.vector.memset(t[:, 64:128], 1.0)
                nc.vector.memset(t[:, 192:256], 1.0)
                vx1r.append(t)

            # ---- phase A: k/v projections + v transpose ----
            for sh in range(2):
                ktl, vtl = {}, {}
                for dc in range(NDC):
                    t = stg.tile([128, 1024], BF16, tag="kv", name="kvstg")
                    nc.gpsimd.dma_start(
                        out=t[:, :],
                        in_=kT[dc * 128:(dc + 1) * 128,
                               sh * 1024:(sh + 1) * 1024])
                    ktl[dc] = t
                for dc in range(NDC):
                    t = stg.tile([128, 1024], BF16, tag="kv", name="kvstg")
                    nc.gpsimd.dma_start(
                        out=t[:, :],
                        in_=vT[dc * 128:(dc + 1) * 128,
                               sh * 1024:(sh + 1) * 1024])
                    vtl[dc] = t
                for ss in (2 * sh, 2 * sh + 1):
                    lo = (ss % 2) * 512
                    ps = ps_pj.tile([128, 512], F32, tag="pj")
                    for dc in range(NDC):
                        nc.tensor.matmul(ps[:, :], wk_t[dc][:, :],
                                         ktl[dc][:, lo:lo + 512],
                                         start=(dc == 0),
                                         stop=(dc == NDC - 1))
                    nc.vector.tensor_scalar_add(kxT[ss][:, :], ps[:, :],
                                                bk_t[:, 0:1])
                    ps = ps_pj.tile([128, 512], F32, tag="pj")
                    for dc in range(NDC):
                        nc.tensor.matmul(ps[:, :], wv_t[dc][:, :],
                                         vtl[dc][:, lo:lo + 512],
                                         start=(dc == 0),
                                         stop=(dc == NDC - 1))
                    vsb = vxsb.tile([128, 512], BF16, tag="vsb")
                    nc.vector.tensor_scalar_add(vsb[:, :], ps[:, :],
                                                bv_t[:, 0:1])
                    vtp = ps_pj.tile([128, 512], BF16, tag="pj")
                    for jr in range(4):
                        nc.tensor.transpose(vtp[:, jr * 128:(jr + 1) * 128],
                                            vsb[:, jr * 128:(jr + 1) * 128],
                                            id_t[:, :])
                    for jr in range(4):
                        jb = ss * 4 + jr
                        nc.vector.tensor_copy(
                            vx1r[jb][:, 0:64],
                            vtp[:, jr * 128:jr * 128 + 64])
                        nc.vector.tensor_copy(
                            vx1r[jb][:, 128:192],
                            vtp[:, jr * 128 + 64:jr * 128 + 128])

            # ---- phase B: per i-block ----
            for ss in range(NSS):
                s0 = ss * 512
                sh, lo = ss // 2, (ss % 2) * 512
                if ss == 2:
                    load_qT(1)
                # GEMM1 q: qxT for this i-block
                for cc in range(4):
                    ps = ps_pj.tile([128, 512], F32, tag="pj")
                    for dc in range(NDC):
                        nc.tensor.matmul(
                            ps[:, :], wq_t[dc][:, cc * 128:(cc + 1) * 128],
                            qT_t[(dc, sh)][:, lo:lo + 512],
                            start=(dc == 0), stop=(dc == NDC - 1))
                    nc.vector.tensor_scalar_add(qxT[(cc, ss)][:, :],
                                                ps[:, :], bq_t[:, cc:cc + 1])

                # attention: 4 head-pairs (cc, cc+4)
                njb = 4 * (ss + 1) if causal else NJB
                for pair in range(4):
                    qx = qxT[(pair, ss)]
                    at = ps_at.tile([128, 1024], F32, tag="at")
                    for jb in range(njb):
                        jss, jr = jb // 4, jb % 4
                        off = max(0, jb * 128 - s0) if causal else 0
                        N = 512 - off
                        sp = ps_sp.tile([128, 1024], F32, tag="sp")
                        nc.tensor.matmul(
                            sp[:, 0:N],
                            kxT[jss][0:64, jr * 128:(jr + 1) * 128],
                            qx[0:64, off:512], start=True, stop=True)
                        nc.tensor.matmul(
                            sp[:, 512:512 + N],
                            kxT[jss][64:128, jr * 128:(jr + 1) * 128],
                            qx[64:128, off:512], start=True, stop=True)
                        if mode == "dense":
                            mb = mbp.tile([128, 512], F32, tag="mb")
                            nc.sync.dma_start(
                                out=mb[:, 0:N],
                                in_=mbias[jb * 128:(jb + 1) * 128,
                                          s0 + off:s0 + 512])
                            nc.vector.tensor_tensor(
                                sp[:, 0:N], sp[:, 0:N], mb[:, 0:N], ALU.add)
                            nc.vector.tensor_tensor(
                                sp[:, 512:512 + N], sp[:, 512:512 + N],
                                mb[:, 0:N], ALU.add)
                        ex = expp.tile([128, 1024], BF16, tag="ex")
                        nc.scalar.activation(
                            _ap3(ex[:, 0:1024], 512, 2, N),
                            _ap3(sp[:, 0:1024], 512, 2, N),
                            AF.Exp, scale=0.125)
                        if causal and jss == ss:
                            nc.vector.tensor_tensor(
                                _ap3(ex[:, 0:1024], 512, 2, 128),
                                _ap3(ex[:, 0:1024], 512, 2, 128),
                                _ap3(tri2_t[:, 0:256], 128, 2, 128),
                                ALU.mult)
                        nc.tensor.matmul(
                            at[:, off:512], vx1r[jb][:, 0:128],
                            ex[:, 0:N],
                            start=(jb == 0), stop=(jb == njb - 1))
                        nc.tensor.matmul(
                            at[:, 512 + off:1024], vx1r[jb][:, 128:256],
                            ex[:, 512:512 + N],
                            start=(jb == 0), stop=(jb == njb - 1))
                    # normalize: rows 64:128 hold the denominator,
                    # replicated 64-wide by the ones columns
                    # 1/D = exp(-ln(D)) on ACT: Exp and Ln share one
                    # table set, and each call is ~6x cheaper than DVE
                    # InstReciprocal
                    lnD = nmp.tile([64, 1024], F32, tag="nm")
                    nc.scalar.activation(lnD[:, :], at[64:128, 0:1024], AF.Ln)
                    nm = nmp.tile([64, 1024], F32, tag="nm")
                    nc.scalar.activation(nm[:, :], lnD[:, :], AF.Exp,
                                         scale=-1.0)
                    aT = attnT[(pair, ss)]
                    nc.vector.tensor_tensor(
                        aT[0:64, :], at[0:64, 0:512], nm[0:64, 0:512],
                        ALU.mult)
                    nc.vector.tensor_tensor(
                        aT[64:128, :], at[0:64, 512:1024],
                        nm[0:64, 512:1024], ALU.mult)

                # GEMM4: output projection (fp32 partial)
                for ic in range(4):
                    i0 = ic * 128
                    for hf in range(2):
                        ob = obp.tile([128, 1024], F32, tag="ob")
                        for e2 in range(2):
                            ec = hf * 2 + e2
                            g4 = ps_pj.tile([128, 512], F32, tag="pj")
                            for cc2 in range(4):
                                nc.tensor.matmul(
                                    g4[:, :],
                                    attnT[(cc2, ss)][:, i0:i0 + 128],
                                    wo_t[cc2][:, ec * 512:(ec + 1) * 512],
                                    start=(cc2 == 0), stop=(cc2 == 3))
                            nc.vector.tensor_copy(
                                ob[:, e2 * 512:(e2 + 1) * 512], g4[:, :])
                        nc.sync.dma_start(
                            out=out[s0 + i0:s0 + i0 + 128,
                                    hf * 1024:(hf + 1) * 1024],
                            in_=ob[:, :])
    nc.finalize()
    return nc


_CACHE = {}


def _get_nc(mode):
    if mode not in _CACHE:
        _CACHE[mode] = build_nc(mode)
    return _CACHE[mode]


def kernel(q, k, v, mask, Wq, bq, Wk, bk, Wv, bv, Wo, bo):
    bf = ml_dtypes.bfloat16
    q = np.asarray(q, np.float32)
    k = np.asarray(k, np.float32)
    v = np.asarray(v, np.float32)
    mask = np.asarray(mask)
    Wq = np.asarray(Wq, np.float32)
    Wk = np.asarray(Wk, np.float32)
    Wv = np.asarray(Wv, np.float32)
    Wo = np.asarray(Wo, np.float32)
    bq = np.asarray(bq, np.float32)
    bk = np.asarray(bk, np.float32)
    bv = np.asarray(bv, np.float32)
    bo = np.asarray(bo, np.float32)

    m = mask.astype(np.float64)
    if np.array_equal(m, np.tril(np.ones((S, S)))):
        mode = "causal"
    elif np.all(m == 1):
        mode = "none"
    else:
        mode = "dense"

    nc = _get_nc(mode)
    tri = np.triu(np.ones((128, 128), np.float32))
    tri2_np = np.concatenate([tri, tri], axis=1).astype(bf)
    id_np = np.eye(128).astype(bf)

    # On-chip layout places local q head h in tile h%4 at partition
    # (h//4)*64 so q/k partition bases match in the scores matmul. Permute
    # Wq columns / Wo rows / bq accordingly: tile cc holds heads (cc, cc+4).
    head_perm = [h for cc in range(4) for h in (cc, cc + 4)]
    col_perm = np.concatenate(
        [np.arange(h * HD, (h + 1) * HD) for h in head_perm])

    # per-batch transposed bf16 inputs (shared across the 4 kv shards)
    qT_b = [np.ascontiguousarray(q[b].astype(bf).T) for b in range(B)]
    kT_b = [np.ascontiguousarray(k[b].astype(bf).T) for b in range(B)]
    vT_b = [np.ascontiguousarray(v[b].astype(bf).T) for b in range(B)]

    in_maps = []
    for core in range(NCORES):
        b, kb = core // KVSH, core % KVSH
        wq_sh = Wq[:, kb * CQ:(kb + 1) * CQ][:, col_perm]
        wo_sh = Wo[kb * CQ:(kb + 1) * CQ, :][col_perm, :]
        bq_sh = bq[kb * CQ:(kb + 1) * CQ][col_perm]
        im = {
            "qT": qT_b[b],
            "kT": kT_b[b],
            "vT": vT_b[b],
            "wq": np.ascontiguousarray(wq_sh.astype(bf)),
            "wk": np.ascontiguousarray(
                Wk[:, kb * CK:(kb + 1) * CK].astype(bf)),
            "wv": np.ascontiguousarray(
                Wv[:, kb * CK:(kb + 1) * CK].astype(bf)),
            "wo": np.ascontiguousarray(wo_sh.astype(bf)),
            "bq": np.ascontiguousarray(bq_sh),
            "bk": np.ascontiguousarray(bk[kb * CK:(kb + 1) * CK]),
            "bv": np.ascontiguousarray(bv[kb * CK:(kb + 1) * CK]),
            "tri2": tri2_np,
            "ident": id_np,
        }
        if mode == "dense":
            with np.errstate(divide="ignore"):
                bias = -(1.0 / mask.astype(np.float32) + 1.0)
            im["mbias"] = np.ascontiguousarray(bias.T * 8.0)
        in_maps.append(im)

    res = run_bass_kernel_spmd(nc, in_maps, core_ids=list(range(NCORES)))
    outs = [r["out"] for r in res.results]
    full = np.empty((B, S, DIM), np.float32)
    for b in range(B):
        acc = outs[b * KVSH].astype(np.float32)
        for kb in range(1, KVSH):
            acc = acc + outs[b * KVSH + kb]
        full[b] = acc + bo[None, :]
    return full
